# revision 40
# baseline (speedup 1.0000x reference)
"""Longformer sliding-window self-attention (B=2, S=4096, D=768, H=12, Dh=64,
one-sided window W=256) on 8 TRN2 NeuronCores.

Sharding: (batch, head-group) — core = b*4 + g handles batch b, heads
[3g, 3g+3). All-bf16 operand path (f32 PSUM accumulation):

  phase 1 (per 512-row s-block, pipelined):
    X^T loaded directly via DMA xbar transpose (host pre-casts X to bf16),
    fused Q/K projection W_qk^T @ X^T (bf16, full 128-row m-tiles),
    V computed row-major (X^T tiles stationary, Wv moving) into V_aug
    [s, kt, h, 65] with a ones column (softmax denominator). Q^T/K^T land
    in per-head zero-padded [128, S] tiles (head data rows 0-63, zeros
    64-127) so the score matmuls run with full K=128 stationary operands
    (fast weight load + LDWEIGHTS/matmul overlap). Row halves that the
    projection emits on PSUM partitions 64-127 are staged and moved down
    by SBUF->SBUF DMA (cross-partition moves need the DMA engines).
  phase 2 (per 256-query chunk, lag-2 software pipeline):
    folded-edge banded scores S^T[k, q]: 4 full key tiles + the two edge
    half-tiles packed into one 256-col slot (slot 0). The K=128
    zero-padded stationary operands keep LDWEIGHTS overlapped with the
    running matmul (~117ns per 256-col score MM vs ~214 at K=64). exp on
    ACT straight out of PSUM, band-edge masking via t_ge/t_le multiplies
    on DVE (bf16 2x mode), AV accumulation with ones-column Z, output
    rows scaled by 1/Z on DVE. Emission order: scores/exp(ci),
    masks/AV(ci-1), recip/epilogue/store(ci-2) — keeps every engine
    queue stall-free.

kernel() takes full inputs, shards, runs SPMD on cores 0..7, reassembles.
"""
import sys

if '/opt/trn_rl_repo' not in sys.path:
    sys.path.insert(0, '/opt/trn_rl_repo')

import math
from contextlib import ExitStack

import numpy as np
import ml_dtypes

import concourse.bacc as bacc
import concourse.mybir as mybir
import concourse.tile as tile
from concourse.bass_utils import run_bass_kernel_spmd

F32 = mybir.dt.float32
BF16 = mybir.dt.bfloat16

B, S, D = 2, 4096, 768
H, DH, W = 12, 64, 256
HPC = 3              # heads per core
DHC = HPC * DH       # 192 head-dims per core
NCORES = 8
C2 = 256             # query chunk
NCH = S // C2        # 16 chunks
NKT = S // 128       # 32 key tiles
SBLK = 512           # projection s-block
NSB = S // SBLK      # 8 s-blocks
VAW = DH + 1         # 65: V columns + ones column
MQK = 2 * DHC        # 384 fused q+k output dims
NSL = 5              # score slots per chunk (slot 0 = folded edges)
SCW = NSL * C2       # 1280 score columns per (chunk, head)
AluOp = mybir.AluOpType
ActFn = mybir.ActivationFunctionType
NPBF16 = ml_dtypes.bfloat16


def _build_program(use_b, use_bv, use_fmask, use_qmask):
    nc = bacc.Bacc("TRN2", num_devices=NCORES)

    xb_d = nc.dram_tensor("xb", (S, D), BF16, kind="ExternalInput").ap()
    wqk_d = nc.dram_tensor("wqk", (D, MQK), BF16, kind="ExternalInput").ap()
    wv_d = nc.dram_tensor("wv", (D, DHC), BF16, kind="ExternalInput").ap()
    tge_d = nc.dram_tensor("t_ge", (128, 128), BF16, kind="ExternalInput").ap()
    tgl_d = nc.dram_tensor("t_gl", (128, 256), BF16, kind="ExternalInput").ap()
    tle_d = nc.dram_tensor("t_le", (128, 128), BF16, kind="ExternalInput").ap()
    if use_b:
        bqk_d = nc.dram_tensor("bqk", (MQK, 1), F32, kind="ExternalInput").ap()
    if use_bv:
        bvr_d = nc.dram_tensor("bvr", (1, DHC), BF16, kind="ExternalInput").ap()
    if use_fmask:
        fmk_d = nc.dram_tensor("fmk", (128, NKT), F32, kind="ExternalInput").ap()
    if use_qmask:
        qmk_d = nc.dram_tensor("qmk", (128, NKT), F32, kind="ExternalInput").ap()
    out_d = nc.dram_tensor("out", (S, DHC), F32, kind="ExternalOutput").ap()

    with tile.TileContext(nc) as tc, ExitStack() as ctx:
        pers = ctx.enter_context(tc.tile_pool(name="pers", bufs=1))

        wqk = pers.tile([128, 6 * MQK], BF16, tag="wqk", name="wqk")
        nc.scalar.dma_start(wqk[:], wqk_d.rearrange("(a p) n -> p a n", p=128))
        wqk3 = wqk.rearrange("p (a n) -> p a n", a=6)
        wv = pers.tile([128, 6 * DHC], BF16, tag="wv", name="wv")
        nc.scalar.dma_start(wv[:], wv_d.rearrange("(a p) n -> p a n", p=128))
        wv3 = wv.rearrange("p (a n) -> p a n", a=6)
        t_ge = pers.tile([128, 128], BF16, tag="t_ge", name="t_ge")
        t_gl = pers.tile([128, 256], BF16, tag="t_gl", name="t_gl")
        t_le = pers.tile([128, 128], BF16, tag="t_le", name="t_le")
        nc.scalar.dma_start(t_ge[:], tge_d)
        nc.scalar.dma_start(t_gl[:], tgl_d)
        nc.scalar.dma_start(t_le[:], tle_d)
        if use_b:
            bqk = pers.tile([128, 3], F32, tag="bqk", name="bqk")
            nc.scalar.dma_start(bqk[:], bqk_d.rearrange("(m p) c -> p (m c)", p=128))
        if use_bv:
            bvr = pers.tile([1, DHC], BF16, tag="bvr", name="bvr")
            nc.scalar.dma_start(bvr[:], bvr_d)
        if use_fmask:
            fmk = pers.tile([128, NKT], F32, tag="fmk", name="fmk")
            nc.scalar.dma_start(fmk[:], fmk_d)
        if use_qmask:
            qmk = pers.tile([128, NKT], F32, tag="qmk", name="qmk")
            nc.scalar.dma_start(qmk[:], qmk_d)

        # persistent activations
        xT = pers.tile([128, 6 * S], BF16, tag="xT", name="xT")
        xT3 = xT.rearrange("p (a s) -> p a s", a=6)
        # per-head zero-padded Q^T/K^T: head data rows 0-63, zeros 64-127
        qz = [pers.tile([128, S], BF16, tag=f"qz{h}", name=f"qz{h}")
              for h in range(HPC)]
        kz = [pers.tile([128, S], BF16, tag=f"kz{h}", name=f"kz{h}")
              for h in range(HPC)]
        # staging for the projection outputs that land on PSUM rows 64-127
        # (q-h1, k-h1, k-h2); moved to rows 0-63 of qz/kz by SBUF->SBUF DMA
        stg = [pers.tile([128, S], BF16, tag=f"stg{i}", name=f"stg{i}")
               for i in range(3)]
        for t in qz + kz:
            nc.gpsimd.memset(t[64:128, :], 0.0)
        va = pers.tile([128, NKT * HPC * VAW], BF16, tag="va", name="va")
        va4 = va.rearrange("p (t h c) -> p t h c", h=HPC, c=VAW)
        nc.gpsimd.memset(va4[:, :, :, DH:VAW], 1.0)
        # pre-warm the exp table while phase 1 runs
        scr = pers.tile([1, 1], BF16, tag="scr", name="scr")
        nc.scalar.activation(scr[:], t_ge[0:1, 0:1], ActFn.Exp)

        # ---------------- phase 1: X^T, projections, V_aug ----------------
        with tc.tile_pool(name="psB", bufs=2, space="PSUM") as psB, \
             tc.tile_pool(name="psC", bufs=4, space="PSUM") as psC:
            for sb in range(NSB):
                ssl = slice(sb * SBLK, (sb + 1) * SBLK)
                if sb % 2 == 0:
                    # one xbar call per (dt, 1024-row slab): amortizes the
                    # ~700ns per-call DGE overhead over twice the tiles
                    psl = slice(sb * SBLK, (sb + 2) * SBLK)
                    for dt in range(6):
                        nc.sync.dma_start_transpose(
                            xT3[:, dt, psl],
                            xb_d[psl, dt * 128:(dt + 1) * 128])
                # fused Q/K projection: m0 = q h0 | q h1, m1 = k h0 | k h1,
                # m2 = q h2 | k h2
                for m in range(3):
                    ps = psB.tile([128, SBLK], F32, tag="pb", name=f"pb{m}_{sb}",
                                  bufs=2)
                    for k in range(6):
                        nc.tensor.matmul(
                            ps[:],
                            wqk3[:, k, m * 128:(m + 1) * 128],
                            xT3[:, k, ssl],
                            start=(k == 0), stop=(k == 5))
                    if m == 0:
                        dsts = [(qz[0][0:64, ssl], ps[0:64, :], (0, 64)),
                                (stg[0][64:128, ssl], ps[64:128, :], (64, 128))]
                    elif m == 1:
                        dsts = [(kz[0][0:64, ssl], ps[0:64, :], (0, 64)),
                                (stg[1][64:128, ssl], ps[64:128, :], (64, 128))]
                    else:
                        dsts = [(qz[2][0:64, ssl], ps[0:64, :], (0, 64)),
                                (stg[2][64:128, ssl], ps[64:128, :], (64, 128))]
                    for dst, src, rows in dsts:
                        if use_b:
                            bias = bqk[rows[0]:rows[1], m:m + 1]
                            nc.vector.tensor_scalar_add(dst, src, bias)
                        else:
                            nc.vector.tensor_copy(dst, src)
                # V row-major into V_aug
                for st in range(sb * 4, sb * 4 + 4):
                    vps = psC.tile([128, DHC], F32, tag="vps", name="vps",
                                   bufs=4)
                    for k in range(6):
                        nc.tensor.matmul(
                            vps[:],
                            xT3[:, k, st * 128:(st + 1) * 128],
                            wv3[:, k, :],
                            start=(k == 0), stop=(k == 5 and not use_bv))
                    if use_bv:
                        # bias via K=1 ones-row matmul (t_le row 0 = ones)
                        nc.tensor.matmul(
                            vps[:], t_le[0:1, 0:128], bvr[0:1, :],
                            start=False, stop=True)
                    nc.vector.tensor_copy(
                        va4[:, st, :, 0:DH],
                        vps.rearrange("p (h d) -> p h d", h=HPC))


            # move staged halves down to rows 0-63 of their padded tiles
            nc.sync.dma_start(qz[1][0:64, :], stg[0][64:128, :])
            nc.sync.dma_start(kz[1][0:64, :], stg[1][64:128, :])
            nc.sync.dma_start(kz[2][0:64, :], stg[2][64:128, :])

        # ---------------- phase 2: banded attention ----------------
        # slot layout per (chunk, head): slot 0 cols [0:128] = edge tile
        # j=-2 half0 (t_ge), cols [128:256] = edge tile j=3 half1 (t_le);
        # slots 1..4 = full tiles j=-1..2.
        def slots_for(ci):
            """[(slot, col0, ncol, kt, half_or_None)]"""
            out = []
            kt_lo = 2 * ci - 2
            if kt_lo >= 0:
                out.append((0, 0, 128, kt_lo, 0))
            kt_hi = 2 * ci + 3
            if kt_hi <= NKT - 1:
                out.append((0, 128, 128, kt_hi, 1))
            for j in range(-1, 3):
                kt = 2 * ci + j
                if 0 <= kt <= NKT - 1:
                    out.append((1 + j + 1, (j + 2) * C2, C2, kt, None))
            return out

        with tc.tile_pool(name="pD", bufs=1) as pD, \
             tc.tile_pool(name="psD_sc", bufs=2, space="PSUM") as psD_sc, \
             tc.tile_pool(name="psD_av", bufs=2, space="PSUM") as psD_av:
            state = {}

            def front(ci):
                qsl = slice(ci * C2, (ci + 1) * C2)
                sl = slots_for(ci)
                scs = [psD_sc.tile([128, SCW], F32, tag="sc",
                                   name=f"sc{ci}_{h}", bufs=2)
                       for h in range(HPC)]
                # head-major emission: each head's exp can start as soon as
                # that head's scores finish, instead of at the end of the
                # whole block — unblocks the next chunk's sc-buffer rotation
                # ~1.2us earlier and removes a per-chunk PE stall
                hi = SCW if 2 * ci + 2 <= NKT - 1 else 4 * C2
                pts = []
                for h in range(HPC):
                    for _, c0, nc_, kt, hf in sl:
                        ksl = slice(kt * 128, (kt + 1) * 128)
                        if hf is None:
                            q0 = qsl
                        else:
                            q0 = slice(ci * C2 + hf * 128,
                                       ci * C2 + hf * 128 + 128)
                        nc.tensor.matmul(scs[h][:, c0:c0 + nc_], kz[h][:, ksl],
                                         qz[h][:, q0], start=True, stop=True)
                    pt = pD.tile([128, SCW], BF16, tag="pt",
                                 name=f"pt{ci}_{h}", bufs=6)
                    nc.scalar.activation(pt[:, 0:hi], scs[h][:, 0:hi],
                                         ActFn.Exp)
                    pts.append(pt)
                state[ci] = pts

            def mid(ci):
                pts = state[ci]
                sl = slots_for(ci)
                has_lo = 2 * ci - 2 >= 0
                has_hi = 2 * ci + 3 <= NKT - 1
                av_big = psD_av.tile([128, 6 * VAW], F32, tag="av",
                                     name=f"av{ci}", bufs=2)
                av6 = av_big.rearrange("p (g c) -> p g c", c=VAW)
                state[(ci, 'av')] = av6
                for h in range(HPC):
                    pt = pts[h]
                    if has_lo and has_hi:
                        nc.vector.tensor_tensor(pt[:, 0:256], pt[:, 0:256],
                                                t_gl[:], op=AluOp.mult)
                    elif has_lo:
                        nc.vector.tensor_tensor(pt[:, 0:128], pt[:, 0:128],
                                                t_ge[:], op=AluOp.mult)
                    elif has_hi:
                        nc.vector.tensor_tensor(pt[:, 128:256], pt[:, 128:256],
                                                t_le[:], op=AluOp.mult)
                    # full-tile edge masks: j=-1 half1 (t_ge), j=2 half0 (t_le)
                    for slot, c0, nc_, kt, hf in sl:
                        j = kt - 2 * ci
                        if hf is not None:
                            continue
                        if j == -1:
                            nc.vector.tensor_tensor(
                                pt[:, c0 + 128:c0 + 256],
                                pt[:, c0 + 128:c0 + 256], t_ge[:],
                                op=AluOp.mult)
                        elif j == 2:
                            nc.vector.tensor_tensor(
                                pt[:, c0:c0 + 128], pt[:, c0:c0 + 128],
                                t_le[:], op=AluOp.mult)
                    if use_fmask:
                        for slot, c0, nc_, kt, hf in sl:
                            nc.vector.tensor_scalar_mul(
                                pt[:, c0:c0 + nc_], pt[:, c0:c0 + nc_],
                                fmk[:, kt:kt + 1])
                    # AV accumulation; mask-free slots first so the group
                    # starts before the edge-mask DVE ops finish
                    for hf in range(2):
                        lst = []
                        for slot, c0, nc_, kt, shf in sl:
                            j = kt - 2 * ci
                            if shf is None:
                                masked = (j == -1 and hf == 1) or \
                                         (j == 2 and hf == 0)
                                lst.append((masked, kt,
                                            pt[:, c0 + hf * 128:
                                               c0 + hf * 128 + 128]))
                            elif shf == hf:
                                lst.append((True, kt, pt[:, c0:c0 + 128]))
                        lst.sort(key=lambda x: (x[0], x[1]))
                        g = h * 2 + hf
                        for i, (msk, kt, psl) in enumerate(lst):
                            nc.tensor.matmul(
                                av6[:, g, :], psl, va4[:, kt, h, :],
                                start=(i == 0), stop=(i == len(lst) - 1))

            def tail(ci):
                av6 = state.pop((ci, 'av'))
                state.pop(ci)
                os_t = [pD.tile([128, DHC], F32, tag="os", name=f"os{ci}_{hf}",
                                bufs=4) for hf in range(2)]
                rzs = pD.tile([128, 6], F32, tag="rzs", name=f"rzs{ci}",
                              bufs=3)
                nc.vector.reciprocal(rzs[:], av6[:, :, DH])
                if use_qmask:
                    for g in range(6):
                        nc.vector.tensor_scalar_mul(
                            rzs[:, g:g + 1], rzs[:, g:g + 1],
                            qmk[:, 2 * ci + (g % 2):2 * ci + (g % 2) + 1])
                for h in range(HPC):
                    for hf in range(2):
                        g = h * 2 + hf
                        nc.vector.tensor_scalar_mul(
                            os_t[hf][:, h * DH:(h + 1) * DH],
                            av6[:, g, 0:DH], rzs[:, g:g + 1])
                for hf in range(2):
                    qt = 2 * ci + hf
                    nc.sync.dma_start(
                        out_d[qt * 128:(qt + 1) * 128, :], os_t[hf][:])

            for ci in range(NCH):
                front(ci)
                if ci >= 1:
                    mid(ci - 1)
                if ci >= 2:
                    tail(ci - 2)
            mid(NCH - 1)
            tail(NCH - 2)
            tail(NCH - 1)

    nc.compile()
    return nc


_prog_cache = {}


def _get_program(use_b, use_bv, use_fmask, use_qmask):
    key = (use_b, use_bv, use_fmask, use_qmask)
    if key not in _prog_cache:
        _prog_cache[key] = _build_program(use_b, use_bv, use_fmask, use_qmask)
    return _prog_cache[key]


def _host_constants():
    kl = np.arange(128)[:, None]
    ql = np.arange(128)[None, :]
    t_ge = (kl >= ql).astype(NPBF16)
    t_le = (kl <= ql).astype(NPBF16)
    t_gl = np.concatenate([t_ge, t_le], axis=1)
    return t_ge, t_gl, t_le


def kernel(hidden_states, attention_mask, is_index_masked, Wq, bq, Wk, bk, Wv, bv,
           trace=False):
    hidden_states = np.asarray(hidden_states, dtype=np.float32)
    attention_mask = np.asarray(attention_mask, dtype=np.float32)
    is_index_masked = np.asarray(is_index_masked)
    Wq = np.asarray(Wq, dtype=np.float32)
    Wk = np.asarray(Wk, dtype=np.float32)
    Wv = np.asarray(Wv, dtype=np.float32)
    bq = np.asarray(bq, dtype=np.float32)
    bk = np.asarray(bk, dtype=np.float32)
    bv = np.asarray(bv, dtype=np.float32)

    use_b = bool(np.any(bq != 0) or np.any(bk != 0))
    use_bv = bool(np.any(bv != 0))
    use_fmask = bool(np.any(attention_mask != 0))
    use_qmask = bool(np.any(is_index_masked))
    nc = _get_program(use_b, use_bv, use_fmask, use_qmask)

    scale = 1.0 / math.sqrt(DH)
    t_ge, t_gl, t_le = _host_constants()

    xb = hidden_states.astype(NPBF16)

    in_maps = []
    for cid in range(NCORES):
        b = cid // 4
        h0 = HPC * (cid % 4)
        c0, c1 = h0 * DH, (h0 + HPC) * DH
        wql = Wq[:, c0:c1] * scale
        wkl = Wk[:, c0:c1]
        wqk_h = np.concatenate(
            [wql[:, 0:128], wkl[:, 0:128], wql[:, 128:192], wkl[:, 128:192]],
            axis=1).astype(NPBF16)
        m = {
            "xb": xb[b],
            "wqk": np.ascontiguousarray(wqk_h),
            "wv": np.ascontiguousarray(Wv[:, c0:c1].astype(NPBF16)),
            "t_ge": t_ge,
            "t_gl": t_gl,
            "t_le": t_le,
        }
        if use_b:
            bql = bq[c0:c1] * scale
            bkl = bk[c0:c1]
            m["bqk"] = np.ascontiguousarray(np.concatenate(
                [bql[0:128], bkl[0:128], bql[128:192], bkl[128:192]])
                .reshape(MQK, 1))
        if use_bv:
            m["bvr"] = np.ascontiguousarray(
                bv[c0:c1].astype(NPBF16).reshape(1, DHC))
        if use_fmask:
            fac = (attention_mask[b] == 0).astype(np.float32)
            m["fmk"] = np.ascontiguousarray(fac.reshape(NKT, 128).T)
        if use_qmask:
            keep = (~is_index_masked[b]).astype(np.float32)
            m["qmk"] = np.ascontiguousarray(keep.reshape(NKT, 128).T)
        in_maps.append(m)

    res = run_bass_kernel_spmd(nc, in_maps, core_ids=list(range(NCORES)),
                               trace=trace)
    out = np.empty((B, S, D), dtype=np.float32)
    for cid in range(NCORES):
        b = cid // 4
        h0 = HPC * (cid % 4)
        out[b, :, h0 * DH:(h0 + HPC) * DH] = res.results[cid]["out"]
    if trace:
        return out, res
    return out
